# revision 31
# baseline (speedup 1.0000x reference)
"""MinLSTM Trainium2 kernel (restructured).

Math (identical to the log-space reference, in linear space):
    sf = sigmoid(x @ W_f.T + b_f)
    si = sigmoid(x @ W_i.T + b_i)
    zh = x @ W_h.T + b_h
    g  = max(zh + 0.5, sigmoid(zh))           (exact rewrite of log_g)
    a  = sf / (sf + si)                        (normalized forget gate)
    b  = si * g / (sf + si)                    (normalized input contribution)
    h_t = a_t * h_{t-1} + b_t                  (hardware tensor_tensor_scan)

All values are positive and O(1) so the linear recurrence is stable.

Sharding: data-parallel over batch B=8, one batch per NeuronCore. Host
pre-transposes x[b] to [D, T] so gate matmuls produce z in [H-partition,
T-free] layout, which the per-partition scan along the free dim needs.

Structure per core:
  - Matmuls run weight-stationary, k-contiguous: for each (hidden block,
    gate, T-quarter) group, the 4 contraction tiles each load weights once
    and stream 2x N=512 moving blocks, accumulating into a [128, 1024]
    PSUM tile (2 banks, 4-deep pool).
  - Gate order per unit is (h, f, i) so the g=max(...) fusion can drain
    zh's PSUM early.
  - ACT does the 3 sigmoids straight out of PSUM (bias fused), bf16 out.
  - DVE: gg=max(zh+bh+.5, th) [PSUM], rr=1/ts, aa=1-qq (4x mode),
    bb=qq*gg (2x mode), plus scans for hb 0-1.
  - Pool: ts=tf+ti and qq=ti*rr as scalar_tensor_tensor (0.6 eff), plus
    scans for hb 2-3.
  - Output ht is bf16 [H, T]; host upcasts to f32 and transposes.

mm="bf16": bf16 matmul operands (full PE rate, half DMA).
mm="fp8":  fp8e4(e4m3) operands with DoubleRow (2x PE rate, quarter DMA).
"""

import os
import sys

for _p in ("/opt/trn_rl_repo", "/root/.axon_site/_ro/trn_rl_repo"):
    if os.path.isdir(_p) and _p not in sys.path:
        sys.path.insert(0, _p)

import numpy as np

import concourse.bacc as bacc
import concourse.tile as tile
from concourse import bass_utils, mybir
from concourse.mybir import ActivationFunctionType as AF
from concourse.mybir import AluOpType as ALU

B, T, D, H = 8, 4096, 512, 512
P = 128
KD = D // P       # 4 contraction blocks
HB = H // P       # 4 hidden-partition blocks
TQ = 1024         # matmul/sigmoid quarter width (2 fp32 PSUM banks)
NQ = T // TQ      # 4 quarters
TU = 2048         # elementwise unit width
NU = T // TU      # 2 units ("halves") per hidden block
F32 = mybir.dt.float32
BF16 = mybir.dt.bfloat16
FP8 = mybir.dt.float8e4

MM = "hybrid"     # "bf16" | "fp8" | "hybrid" (f,i gates fp8; h gate bf16)
OUT_DT = BF16

_CACHE = {}


def _gate_dtypes(mm):
    """Per-gate matmul dtype: gates (0=f, 1=i, 2=h)."""
    if mm == "bf16":
        return {0: BF16, 1: BF16, 2: BF16}
    if mm == "fp8":
        return {0: FP8, 1: FP8, 2: FP8}
    if mm == "hybrid":
        return {0: FP8, 1: FP8, 2: BF16}
    raise ValueError(mm)


def _build(n_cores=B, loop_reps=0, mm=MM, out_dt=OUT_DT, ablate=(),
           ts_eng="vector", qq_eng="gpsimd", aa_eng="scalar",
           bb_eng="vector", scan_split=4):
    gdt = _gate_dtypes(mm)
    any8 = any(d == FP8 for d in gdt.values())
    anyb = any(d == BF16 for d in gdt.values())
    nc = bacc.Bacc("TRN2", target_bir_lowering=False, debug=False,
                   num_devices=n_cores)
    g8 = [g for g in range(3) if gdt[g] == FP8]   # fp8 gates
    gb = [g for g in range(3) if gdt[g] == BF16]  # bf16 gates
    # All tensors partition-major so each load is ONE dma_start:
    #   x8 [p, j, i, t]   with d = (2j+i)*128+p (DoubleRow pairs)
    #   xb [p, k, t]      with d = k*128+p
    #   w8 [p, gi, j, i, h] / wb [p, ki, h]  (gi/ki index into g8/gb lists)
    x8_d = (nc.dram_tensor("xT8", [P, 2, 2, T], FP8, kind="ExternalInput")
            if any8 else None)
    xb_d = (nc.dram_tensor("xTb", [P, KD, T], BF16, kind="ExternalInput")
            if anyb else None)
    w8_d = (nc.dram_tensor("wT8", [P, len(g8), 2, 2, H], FP8,
                           kind="ExternalInput") if any8 else None)
    wb_d = (nc.dram_tensor("wTb", [P, len(gb) * KD, H], BF16,
                           kind="ExternalInput") if anyb else None)
    # 4 bias groups packed per partition: [b_f | b_i | b_h | b_h + 0.5]
    bias_d = nc.dram_tensor("biasp", [P, 4 * HB], F32, kind="ExternalInput")
    h0_d = nc.dram_tensor("h0p", [P, HB], F32, kind="ExternalInput")
    ht_d = nc.dram_tensor("ht", [H, T], out_dt, kind="ExternalOutput")

    with tile.TileContext(nc) as tc:
        with (
            tc.tile_pool(name="xp", bufs=1) as xp,
            tc.tile_pool(name="wp", bufs=1) as wp,
            tc.tile_pool(name="cp", bufs=1) as cp,
            tc.tile_pool(name="ps", bufs=4, space="PSUM") as ps,
            tc.tile_pool(name="gp", bufs=3) as gp,
            tc.tile_pool(name="ep", bufs=3) as ep,
            tc.tile_pool(name="hop", bufs=2) as hop,
        ):
            bias = cp.tile([P, 4 * HB], F32, tag="bias")
            nc.sync.dma_start(bias[:], bias_d.ap())
            h0 = cp.tile([P, HB], F32, tag="h0")
            nc.sync.dma_start(h0[:], h0_d.ap())
            # prime the ACT sigmoid table off the critical path
            warm = cp.tile([P, 1], F32, tag="warm")
            nc.scalar.activation(warm[:], h0[:, 0:1], AF.Sigmoid)

            xt8 = (xp.tile([P, 2, 2, T], FP8, tag="x8", name="x8")
                   if any8 else None)
            xtb = (xp.tile([P, KD, T], BF16, tag="xb", name="xb")
                   if anyb else None)
            wt8 = (wp.tile([P, len(g8), 2, 2, H], FP8, tag="w8", name="w8")
                   if any8 else None)
            wtb = (wp.tile([P, len(gb) * KD, H], BF16, tag="wb", name="wb")
                   if anyb else None)

            def load_x(q):
                tsl = slice(q * TQ, (q + 1) * TQ)
                if any8:
                    nc.sync.dma_start(xt8[:, :, :, tsl],
                                      x8_d.ap()[:, :, :, tsl])
                if anyb:
                    nc.sync.dma_start(xtb[:, :, tsl],
                                      xb_d.ap()[:, :, tsl])

            # DMA in consumption order: weights + first T-quarter of x, then
            # the remaining quarters.
            if any8:
                nc.sync.dma_start(wt8[:], w8_d.ap())
            if anyb:
                nc.sync.dma_start(wtb[:], wb_d.ap())
            for q in range(NQ):
                load_x(q)

            def mm_group(z, g, hb, c0, width):
                """z[128, width] (PSUM) = W_g^T[., hb] @ x[., c0:c0+width]."""
                hsl = slice(hb * P, (hb + 1) * P)
                fp8 = gdt[g] == FP8
                nj = 2 if fp8 else KD
                if "mm1" in ablate:
                    nj = 1
                for j in range(nj):
                    for t2 in range(width // 512):
                        c = c0 + t2 * 512
                        zsl = z[:, t2 * 512:(t2 + 1) * 512]
                        if fp8:
                            nc.tensor.matmul(
                                zsl, wt8[:, g8.index(g), j, :, hsl],
                                xt8[:, j, :, c:c + 512],
                                start=(j == 0), stop=(j == nj - 1),
                                perf_mode=mybir.MatmulPerfMode.DoubleRow)
                        else:
                            nc.tensor.matmul(
                                zsl, wtb[:, gb.index(g) * KD + j, hsl],
                                xtb[:, j, c:c + 512],
                                start=(j == 0), stop=(j == nj - 1))

            def gbias(g, hb):
                return bias[:, g * HB + hb:g * HB + hb + 1]

            def phase_a(c0, w, hb):
                """zf/zi matmuls -> sigmoids -> ts -> rr for x cols
                [c0, c0+w). Returns state for phase_b (runs one unit later,
                software-pipelined)."""
                tf = gp.tile([P, w], BF16, tag="tf")
                ti = gp.tile([P, w], BF16, tag="ti")
                for g, gt in ((0, tf), (1, ti)):
                    z = ps.tile([P, w], F32, tag="z")
                    if "mm" not in ablate:
                        mm_group(z, g, hb, c0, w)
                    nc.scalar.activation(gt[:], z[:], AF.Sigmoid,
                                         bias=gbias(g, hb))
                if "nodiv" in ablate:
                    return (c0, w, hb, ti, None)
                # GpSimd's Q7 library only implements plain TensorTensor
                # (no TensorScalarPtr variants), so ts/qq go there as TT.
                ts = ep.tile([P, w], F32, tag="ts")
                te = (("gpsimd" if (c0 // TQ + hb) % 2 else "vector")
                      if ts_eng == "alt" else ts_eng)
                getattr(nc, te).tensor_tensor(
                    ts[:], tf[:], ti[:], ALU.add)
                rr = ep.tile([P, w], F32, tag="rr")
                nc.vector.reciprocal_approx_fast(rr[:], ts[:])
                return (c0, w, hb, ti, rr)

            def phase_b(st, prev):
                """zh matmul -> sigmoid -> gg -> qq -> aa/bb -> scan -> out."""
                c0, w, hb, ti, rr = st
                z = ps.tile([P, w], F32, tag="z")
                if "mm" not in ablate:
                    mm_group(z, 2, hb, c0, w)
                th = gp.tile([P, w], BF16, tag="th")
                nc.scalar.activation(th[:], z[:], AF.Sigmoid,
                                     bias=gbias(2, hb))
                # gg = max(zh + bh + 0.5, sigmoid(zh + bh))
                gg = gp.tile([P, w], BF16, tag="gg")
                nc.vector.scalar_tensor_tensor(
                    gg[:], z[:], bias[:, 3 * HB + hb:3 * HB + hb + 1],
                    th[:], ALU.add, ALU.max)
                if "nodiv" in ablate:
                    aa, bb = ti, gg
                else:
                    qq = ep.tile([P, w], BF16, tag="qq")
                    getattr(nc, qq_eng).tensor_tensor(
                        qq[:], rr[:], ti[:], ALU.mult)
                    aa = ep.tile([P, w], BF16, tag="aa")
                    if aa_eng == "scalar":
                        nc.scalar.activation(aa[:], qq[:], AF.Identity,
                                             bias=1.0, scale=-1.0)
                    else:
                        nc.vector.tensor_scalar(aa[:], qq[:], -1.0, 1.0,
                                                ALU.mult, ALU.add)
                    bb = ep.tile([P, w], BF16, tag="bb")
                    getattr(nc, bb_eng).tensor_tensor(
                        bb[:], qq[:], gg[:], ALU.mult)

                if "noscan" in ablate:
                    ho = bb
                else:
                    ho = hop.tile([P, w], out_dt, tag=f"ho{hb}")
                    if prev[hb] is None:
                        init = h0[:, hb:hb + 1]
                    else:
                        pho, pw = prev[hb]
                        init = pho[:, pw - 1:pw]
                    eng = nc.vector if hb < scan_split else nc.gpsimd
                    eng.tensor_tensor_scan(ho[:], aa[:], bb[:], init,
                                           ALU.mult, ALU.add)
                    prev[hb] = (ho, w)
                if "nodma" not in ablate:
                    nc.sync.dma_start(
                        ht_d.ap()[hb * P:(hb + 1) * P, c0:c0 + w], ho[:])

            import contextlib
            loop_cm = (tc.For_i(0, loop_reps, 1) if loop_reps
                       else contextlib.nullcontext())
            with loop_cm:
                prev = [None] * HB
                pend = None
                for q in range(NQ):
                    for hb in range(HB):
                        # split the final units into 512-wide sub-chunks to
                        # shorten the dependent tail after the last matmul
                        subs = 2 if (q == NQ - 1 and hb >= 2) else 1
                        w = TQ // subs
                        for s in range(subs):
                            st = phase_a(q * TQ + s * w, w, hb)
                            if pend is not None:
                                phase_b(pend, prev)
                            pend = st
                phase_b(pend, prev)

    nc.compile()
    return nc


def _get_module():
    if "nc" not in _CACHE:
        _CACHE["nc"] = _build()
    return _CACHE["nc"]


class _Runner:
    """Caches a compiled 8-core shard_map'd PJRT executable of the Bass
    module so repeat kernel() calls skip jax retracing/compilation."""

    def __init__(self, nc):
        import jax
        from jax.experimental.shard_map import shard_map
        from jax.sharding import Mesh, PartitionSpec

        from concourse import bass2jax

        bass2jax.install_neuronx_cc_hook()
        self.nc = nc
        partition_name = (nc.partition_id_tensor.name
                          if nc.partition_id_tensor else None)
        in_names, out_names, out_avals = [], [], []
        for alloc in nc.m.functions[0].allocations:
            if not isinstance(alloc, mybir.MemoryLocationSet):
                continue
            name = alloc.memorylocations[0].name
            if alloc.kind == "ExternalInput":
                if name != partition_name:
                    in_names.append(name)
            elif alloc.kind == "ExternalOutput":
                out_names.append(name)
                out_avals.append(jax.core.ShapedArray(
                    tuple(alloc.tensor_shape), mybir.dt.np(alloc.dtype)))
        self.in_names = in_names
        self.out_names = out_names
        self.out_avals = out_avals
        n_params, n_outs = len(in_names), len(out_names)
        all_names = list(in_names) + list(out_names)
        if partition_name is not None:
            all_names.append(partition_name)

        def _body(*args):
            operands = list(args)
            if partition_name is not None:
                operands.append(bass2jax.partition_id_tensor())
            return tuple(bass2jax._bass_exec_p.bind(
                *operands,
                out_avals=tuple(out_avals),
                in_names=tuple(all_names),
                out_names=tuple(out_names),
                lowering_input_output_aliases=(),
                sim_require_finite=True,
                sim_require_nnan=True,
                nc=nc,
            ))

        devices = jax.devices()[:B]
        mesh = Mesh(np.asarray(devices), ("core",))
        specs = (PartitionSpec("core"),) * (n_params + n_outs)
        out_specs = (PartitionSpec("core"),) * n_outs
        donate = tuple(range(n_params, n_params + n_outs))
        self._jitted = jax.jit(
            shard_map(_body, mesh=mesh, in_specs=specs,
                      out_specs=out_specs, check_rep=False),
            donate_argnums=donate, keep_unused=True)
        self._compiled = None

    def concat_args(self, in_maps):
        concat_in = [
            np.concatenate([np.asarray(m[name]) for m in in_maps], axis=0)
            for name in self.in_names
        ]
        concat_zeros = [
            np.zeros((B * a.shape[0], *a.shape[1:]), a.dtype)
            for a in self.out_avals
        ]
        return concat_in + concat_zeros

    def compiled(self, args):
        if self._compiled is None:
            self._compiled = self._jitted.lower(*args).compile()
        return self._compiled

    def __call__(self, in_maps):
        import jax
        args = self.concat_args(in_maps)
        outs = jax.block_until_ready(self.compiled(args)(*args))
        return [
            {name: np.asarray(outs[i]).reshape(B, *self.out_avals[i].shape)[c]
             for i, name in enumerate(self.out_names)}
            for c in range(B)
        ]


def _get_runner():
    if "runner" not in _CACHE:
        _CACHE["runner"] = _Runner(_get_module())
    return _CACHE["runner"]


def make_in_maps(x, h_0, W_f, b_f, W_i, b_i, W_h, b_h, mm=MM):
    gdt = _gate_dtypes(mm)
    any8 = any(d == FP8 for d in gdt.values())
    anyb = any(d == BF16 for d in gdt.values())
    f8 = mybir.dt.np(FP8)
    bf = mybir.dt.np(BF16)
    x = np.asarray(x, np.float32)
    h_0 = np.asarray(h_0, np.float32)
    wT = np.ascontiguousarray(
        np.stack([np.asarray(W_f), np.asarray(W_i), np.asarray(W_h)])
        .astype(np.float32).transpose(0, 2, 1))          # [3, D, H]
    b_h = np.asarray(b_h)
    biasp = np.ascontiguousarray(
        np.stack([np.asarray(b_f), np.asarray(b_i), b_h, b_h + 0.5])
        .astype(np.float32).reshape(4, HB, P).transpose(2, 0, 1)
        .reshape(P, 4 * HB))
    g8 = [g for g in range(3) if gdt[g] == FP8]
    gb = [g for g in range(3) if gdt[g] == BF16]
    common = {"biasp": biasp}
    if any8:
        # [3, D, H] -> [p, gi, j, i, H] with d = (2j+i)*128+p
        w8 = wT[g8].reshape(len(g8), 2, 2, P, H)         # [gi, j, i, p, H]
        common["wT8"] = np.ascontiguousarray(
            w8.transpose(3, 0, 1, 2, 4)).astype(f8)
    if anyb:
        # [3, D, H] -> [p, ki, H] with d = k*128+p, ki = gb_idx*KD + k
        wb = wT[gb].reshape(len(gb), KD, P, H)           # [gi, k, p, H]
        common["wTb"] = np.ascontiguousarray(
            wb.transpose(2, 0, 1, 3).reshape(P, len(gb) * KD, H)).astype(bf)
    in_maps = []
    for b in range(B):
        xT = np.ascontiguousarray(x[b].T)                # [D, T]
        m = dict(common)
        if any8:
            # [D, T] -> [p, j, i, T]
            m["xT8"] = np.ascontiguousarray(
                xT.reshape(2, 2, P, T).transpose(2, 0, 1, 3)).astype(f8)
        if anyb:
            # [D, T] -> [p, k, T]
            m["xTb"] = np.ascontiguousarray(
                xT.reshape(KD, P, T).transpose(1, 0, 2)).astype(bf)
        m["h0p"] = np.ascontiguousarray(h_0[b].reshape(HB, P).T)
        in_maps.append(m)
    return in_maps


def kernel(x, h_0, W_f, b_f, W_i, b_i, W_h, b_h):
    in_maps = make_in_maps(x, h_0, W_f, b_f, W_i, b_i, W_h, b_h)
    results = _get_runner()(in_maps)
    out = np.empty((B, T, H), np.float32)
    for b in range(B):
        out[b] = results[b]["ht"].astype(np.float32).T
    return out
